# revision 19
# baseline (speedup 1.0000x reference)
"""Multi-head attention (B=4, S=2048, D=1024, H=16) on 8 Trainium2 cores.

Sharding: data-parallel over the 4 batches x tensor-parallel over 2 groups
of 8 heads. Core c handles batch c//2, head group c%2. Each core computes
its group's slice of the out-projection; the host sums the two partial
outputs per batch.

Device-side layout (per core, all matmul operands float32r):
  qhT/khT [512, S] : projections in transposed layout (head dim on
                     partitions, sequence on free dim), computed with the
                     weight slice as the stationary matmul operand.
  scoresT [sj, si] : per head, via lhsT=khT slice (K=64); softmax keys on
                     partitions so exp rides ScalarE out of PSUM and the
                     denominators come from an all-ones column appended to V
                     during the attn@V accumulation.
  outT [1024, S]   : transposed partial out-projection, summed on host.
"""
import sys

for _p in ("/opt/trn_rl_repo", "/root/.axon_site/_ro/trn_rl_repo"):
    if _p not in sys.path:
        sys.path.append(_p)

import numpy as np

import concourse.bass as bass
import concourse.tile as tile
from concourse import bacc, mybir
from concourse.bass_utils import run_bass_kernel_spmd

N_CORES = 8
B, S, DIM, H, DK = 4, 2048, 1024, 16, 64
JG = DIM // 2          # head-group width (8 heads x 64)
HPG = 8                # heads per group
F32R = mybir.dt.float32r
F32 = mybir.dt.float32

N_KC = DIM // 128      # contraction chunks for projections
N_JT = JG // 128       # 128-row tiles of the group width
N_SJT = S // 128       # key tiles
N_SIC = S // 512       # query chunks
SJ_GRP = 3             # score/exp group size (PSUM banks per group)


def build_program(phases="ABC"):
    nc = bacc.Bacc("TRN2", target_bir_lowering=False, debug=False,
                   num_devices=N_CORES)
    xqT = nc.dram_tensor("xqT", [2, N_KC, 128, 1024], F32R,
                         kind="ExternalInput").ap()
    xkT = nc.dram_tensor("xkT", [2, N_KC, 128, 1024], F32R,
                         kind="ExternalInput").ap()
    xvT = nc.dram_tensor("xvT", [2, N_KC, 128, 1024], F32R,
                         kind="ExternalInput").ap()
    wqT = nc.dram_tensor("wqT", [128, N_KC, JG], F32R,
                         kind="ExternalInput").ap()
    wkT = nc.dram_tensor("wkT", [128, N_KC, JG], F32R,
                         kind="ExternalInput").ap()
    wvT = nc.dram_tensor("wvT", [128, N_KC, JG], F32R,
                         kind="ExternalInput").ap()
    woT = nc.dram_tensor("woT", [128, N_JT, DIM], F32R,
                         kind="ExternalInput").ap()
    bq = nc.dram_tensor("bq", [128, N_JT], F32, kind="ExternalInput").ap()
    bk = nc.dram_tensor("bk", [128, N_JT], F32, kind="ExternalInput").ap()
    bvr = nc.dram_tensor("bvr", [128, JG], F32, kind="ExternalInput").ap()
    outT = nc.dram_tensor("outT", [DIM // 128, N_SIC, 128, 512], F32,
                          kind="ExternalOutput").ap()

    with tile.TileContext(nc) as tc:
        with (
            tc.tile_pool(name="wproj", bufs=2) as wpool,
            tc.tile_pool(name="wo", bufs=1) as wopool,
            tc.tile_pool(name="xin", bufs=3) as xpool,
            tc.tile_pool(name="bias", bufs=1) as bpool,
            tc.tile_pool(name="qk", bufs=1) as qkpool,
            tc.tile_pool(name="vp", bufs=1) as vpool,
            tc.tile_pool(name="attn", bufs=2) as apool,
            tc.tile_pool(name="exp", bufs=2) as epool,
            tc.tile_pool(name="small", bufs=3) as spool,
            tc.tile_pool(name="outsb", bufs=3) as opool,
        ):
            # ---- persistent SBUF residents ----
            qhT = qkpool.tile([128, N_JT, S], F32R, tag="qhT")
            khT = qkpool.tile([128, N_JT, S], F32R, tag="khT")
            v_sb = vpool.tile([128, N_SJT, HPG, DK + 1], F32R, tag="v")
            wo_sb = wopool.tile([128, N_JT, DIM], F32R, tag="wo")
            bq_sb = bpool.tile([128, N_JT], F32, tag="bq")
            bk_sb = bpool.tile([128, N_JT], F32, tag="bk")
            bvr_sb = bpool.tile([128, JG], F32, tag="bvr")

            wk_sb = wpool.tile([128, N_KC, JG], F32R, tag="w", name="wk_sb")
            wv_sb = wpool.tile([128, N_KC, JG], F32R, tag="w", name="wv_sb")
            wq_sb = wpool.tile([128, N_KC, JG], F32R, tag="w", name="wq_sb")
            # wk split per contraction chunk: the first k matmul only needs
            # chunk 0, so it unblocks after 256KB instead of 2MB
            for _kc in range(N_KC):
                nc.scalar.dma_start(wk_sb[:, _kc, :], wkT[:, _kc, :])
            nc.scalar.dma_start(wv_sb[:], wvT[:])
            nc.scalar.dma_start(wq_sb[:], wqT[:])
            nc.sync.dma_start(bq_sb[:], bq[:])
            nc.sync.dma_start(bk_sb[:], bk[:])
            nc.sync.dma_start(bvr_sb[:], bvr[:])
            # ones column for the softmax denominators
            nc.vector.memset(v_sb[:, :, :, DK:DK + 1].bitcast(F32), 1.0)
            # touch Exp early so the ACT table set loads during phase A
            # instead of stalling the first real exp in phase B
            warm = bpool.tile([1, 2], F32, tag="warm")
            nc.vector.memset(warm[:], 0.0)
            nc.scalar.activation(warm[:], warm[:],
                                 mybir.ActivationFunctionType.Exp)

            # ---- phase A: projections ----
            if "A" in phases:
             with tc.tile_pool(name="psA", bufs=8, space="PSUM") as psA:
                # k projection first (phase B needs all of khT), then v,
                # then q (phase B sic0 can start after q's first half)
                for w_sb, x_dram, out_sb, b_sb in (
                    (wk_sb, xkT, khT, bk_sb),
                ):
                    for sh in range(2):
                        ps = [psA.tile([128, 512], F32, tag="ps", name=f"psA{i}")
                              for i in range(8)]
                        for kc in range(N_KC):
                            xt = xpool.tile([128, 1024], F32R, tag="x")
                            nc.sync.dma_start(xt[:], x_dram[sh, kc])
                            for jt in range(N_JT):
                                for sc in range(2):
                                    nc.tensor.matmul(
                                        ps[jt * 2 + sc][:],
                                        w_sb[:, kc, jt * 128:(jt + 1) * 128],
                                        xt[:, sc * 512:(sc + 1) * 512],
                                        start=(kc == 0), stop=(kc == N_KC - 1))
                        for jt in range(N_JT):
                            for sc in range(2):
                                nc.vector.tensor_scalar_add(
                                    out_sb[:, jt,
                                           sh * 1024 + sc * 512:
                                           sh * 1024 + (sc + 1) * 512],
                                    ps[jt * 2 + sc][:],
                                    b_sb[:, jt:jt + 1])

                # v projection, normal layout [s, j] (before q so attention
                # can start as soon as q's first half lands)
                w_sb = wv_sb
                for sh in range(2):
                    ps = [psA.tile([128, 512], F32, tag="ps", name=f"psA{i}")
                          for i in range(8)]
                    for kc in range(N_KC):
                        xt = xpool.tile([128, 1024], F32R, tag="x")
                        nc.sync.dma_start(xt[:], xvT[sh, kc])
                        for st8 in range(8):
                            nc.tensor.matmul(
                                ps[st8][:],
                                xt[:, st8 * 128:(st8 + 1) * 128],
                                w_sb[:, kc, :],
                                start=(kc == 0), stop=(kc == N_KC - 1))
                    for st8 in range(8):
                        st = sh * 8 + st8
                        nc.vector.tensor_tensor(
                            v_sb[:, st, :, 0:DK],
                            ps[st8][:].rearrange("p (h d) -> p h d", h=HPG),
                            bvr_sb[:].rearrange("p (h d) -> p h d", h=HPG),
                            mybir.AluOpType.add)

                # q projection (same structure as k)
                for w_sb, x_dram, out_sb, b_sb in (
                    (wq_sb, xqT, qhT, bq_sb),
                ):
                    for sh in range(2):
                        ps = [psA.tile([128, 512], F32, tag="ps", name=f"psA{i}")
                              for i in range(8)]
                        for kc in range(N_KC):
                            xt = xpool.tile([128, 1024], F32R, tag="x")
                            nc.sync.dma_start(xt[:], x_dram[sh, kc])
                            for jt in range(N_JT):
                                for sc in range(2):
                                    nc.tensor.matmul(
                                        ps[jt * 2 + sc][:],
                                        w_sb[:, kc, jt * 128:(jt + 1) * 128],
                                        xt[:, sc * 512:(sc + 1) * 512],
                                        start=(kc == 0), stop=(kc == N_KC - 1))
                        for jt in range(N_JT):
                            for sc in range(2):
                                nc.vector.tensor_scalar_add(
                                    out_sb[:, jt,
                                           sh * 1024 + sc * 512:
                                           sh * 1024 + (sc + 1) * 512],
                                    ps[jt * 2 + sc][:],
                                    b_sb[:, jt:jt + 1])

            # wo is first needed in phase C — load it after the projection
            # weights so it does not delay the first matmuls
            nc.scalar.dma_start(wo_sb[:], woT[:])

            # ---- phases B/C: attention + out-projection, per query chunk ----
            if "B" in phases:
             with (
                tc.tile_pool(name="psS", bufs=2, space="PSUM") as psS,
                tc.tile_pool(name="psAt", bufs=1, space="PSUM") as psAt,
                tc.tile_pool(name="psO", bufs=1, space="PSUM") as psO,
            ):
                # sjt group boundaries, e.g. [0, 3, 6, 9, 12, 15, 16]
                grp = list(range(0, N_SJT, SJ_GRP)) + [N_SJT]

                def outproj(at_tile, sic_idx, ct):
                    po_c = psO.tile([128, 512], F32, tag="po", name="po_c")
                    for jc in range(N_JT):
                        nc.tensor.matmul(
                            po_c[:],
                            wo_sb[:, jc, ct * 128:(ct + 1) * 128],
                            at_tile[:, jc, :],
                            start=(jc == 0), stop=(jc == N_JT - 1))
                    ob = opool.tile([128, 512], F32, tag="ob", name="ob")
                    nc.vector.tensor_copy(ob[:], po_c[:])
                    nc.sync.dma_start(outT[ct, sic_idx], ob[:])

                at_prev = None
                for sic in range(N_SIC):
                    si = slice(sic * 512, (sic + 1) * 512)
                    at_sb = apool.tile([128, N_JT, 512], F32R, tag="at")
                    for h in range(HPG):
                        jt, po = h // 2, (h % 2) * 64
                        pa = psAt.tile([DK + 1, 512], F32, tag="pa")
                        for gi in range(len(grp) - 1):
                            g0, g1 = grp[gi], grp[gi + 1]
                            gn = g1 - g0
                            ps = psS.tile([128, SJ_GRP, 512], F32, tag="ps")
                            for i in range(gn):
                                sjt = g0 + i
                                nc.tensor.matmul(
                                    ps[:, i, :],
                                    khT[po:po + 64, jt,
                                        sjt * 128:(sjt + 1) * 128],
                                    qhT[po:po + 64, jt, si],
                                    start=True, stop=True)
                            et = epool.tile([128, SJ_GRP, 512], F32R, tag="e")
                            nc.scalar.activation(
                                et[:, :gn, :], ps[:, :gn, :],
                                mybir.ActivationFunctionType.Exp,
                                scale=1.0 / np.sqrt(DK))
                            for i in range(gn):
                                sjt = g0 + i
                                nc.tensor.matmul(
                                    pa[:], v_sb[:, sjt, h, :], et[:, i, :],
                                    start=(sjt == 0), stop=(sjt == N_SJT - 1))
                        den = spool.tile([1, 512], F32, tag="den")
                        nc.vector.reciprocal(den[:], pa[DK:DK + 1, :])
                        bc = spool.tile([DK, 512], F32, tag="bc")
                        nc.gpsimd.partition_broadcast(bc[:], den[:])
                        nc.vector.tensor_tensor(
                            at_sb[po:po + 64, jt, :], pa[:DK, :], bc[:],
                            mybir.AluOpType.mult)
                        # previous chunk's out-projection rides under this
                        # chunk's exp work (PE fills ACT-bound gaps)
                        if "C" in phases and at_prev is not None and h < 4:
                            outproj(at_prev, sic - 1, 2 * h)
                            outproj(at_prev, sic - 1, 2 * h + 1)
                    at_prev = at_sb
                if "C" in phases and at_prev is not None:
                    for ct in range(DIM // 128):
                        outproj(at_prev, N_SIC - 1, ct)
            if "B" not in phases:
                with tc.tile_pool(name="fb", bufs=1) as fb:
                    t0 = fb.tile([128, 512], F32)
                    nc.vector.memset(t0[:], 0.0)
                    nc.sync.dma_start(outT[0:128, 0:512], t0[:])
    nc.compile()
    return nc


_CACHED_NC = None


def _get_program():
    global _CACHED_NC
    if _CACHED_NC is None:
        _CACHED_NC = build_program()
    return _CACHED_NC


def _make_in_maps(q, k, v, Wq, bq, Wk, bk, Wv, bv, Wo, bo):
    f32 = np.float32

    def chunk_x(x):
        # [S, DIM] -> transposed, pre-chunked [2, N_KC, 128, 1024]
        xT = np.asarray(x, f32).T          # [DIM, S]
        return np.ascontiguousarray(
            xT.reshape(N_KC, 128, 2, 1024).transpose(2, 0, 1, 3))

    in_maps = []
    # per-batch transposed activations (shared between the 2 TP cores)
    xT = {}
    for b in range(B):
        xT[b] = (chunk_x(q[b]), chunk_x(k[b]), chunk_x(v[b]))
    wg = {}
    for g in range(2):
        js = slice(g * JG, (g + 1) * JG)
        def tile_w(W):
            # W[js, :].T = [DIM, JG] -> [128, N_KC, JG]
            wT = np.asarray(W, f32)[js, :].T
            return np.ascontiguousarray(
                wT.reshape(N_KC, 128, JG).transpose(1, 0, 2))

        woT_g = np.asarray(Wo, f32)[:, js].T   # [JG, DIM]
        wg[g] = {
            "wqT": tile_w(Wq),
            "wkT": tile_w(Wk),
            "wvT": tile_w(Wv),
            "woT": np.ascontiguousarray(
                woT_g.reshape(N_JT, 128, DIM).transpose(1, 0, 2)),
            "bq": np.ascontiguousarray(
                np.asarray(bq, f32)[js].reshape(N_JT, 128).T),
            "bk": np.ascontiguousarray(
                np.asarray(bk, f32)[js].reshape(N_JT, 128).T),
            "bvr": np.ascontiguousarray(
                np.broadcast_to(np.asarray(bv, f32)[js], (128, JG))),
        }
    for c in range(N_CORES):
        b, g = c // 2, c % 2
        m = {"xqT": xT[b][0], "xkT": xT[b][1], "xvT": xT[b][2]}
        m.update(wg[g])
        in_maps.append(m)
    return in_maps


def _gather(results, bo):
    out = np.empty((B, S, DIM), np.float32)
    bo32 = np.asarray(bo, np.float32)
    for b in range(B):
        acc = results[2 * b]["outT"] + results[2 * b + 1]["outT"]
        # [ct, sic, p, s'] -> [DIM, S]
        full = acc.transpose(0, 2, 1, 3).reshape(DIM, S)
        out[b] = full.T + bo32
    return out


def kernel(q, k, v, Wq, bq, Wk, bk, Wv, bv, Wo, bo):
    import time as _time
    nc = _get_program()
    in_maps = _make_in_maps(q, k, v, Wq, bq, Wk, bk, Wv, bv, Wo, bo)
    last_err = None
    for attempt in range(3):
        try:
            res = run_bass_kernel_spmd(nc, in_maps,
                                       core_ids=list(range(N_CORES)))
            return _gather(res.results, bo)
        except Exception as e:  # transient device/tunnel errors
            last_err = e
            _time.sleep(20 * (attempt + 1))
    raise last_err
